# revision 22
# baseline (speedup 1.0000x reference)
"""Trainium2 Bass kernel: masked multi-head decode attention + output projection.

Problem (hardcoded): query [256,1,512] f32, key/value [256,2048,512] f32,
W_o [512,512] f32, mask [256,1,2048] bool (True = excluded).
out = Linear(W_o) o MHA(query, key, value, mask), 8 heads, dh=64.

Strategy: data-parallel over batch on 8 NeuronCores (32 batches/core).
~50% of keys are masked out and contribute nothing, so the host packs only
the unmasked K/V rows per batch (padded to KPAD=1152 = 9 tiles of 128) and
stores BOTH K and V in fp8 e3m4 — ~7x less HBM traffic than the dense f32
layout. K and V for one batch travel in a single combined DMA.

Per batch on-core:
  - kv[p, t, 0:512]  = K tile t, c-major chunks: kv[p,t,c*128+j] = K[key
    t*128+j, dim c*128+p]; kv[p, t, 512:1024] = V row t*128+p.
  - scores: s_all[k, j*8+h] accumulated over 4 chunk matmuls per key tile
    with the 128-col K chunk stationary (fp8 FWL load) and the
    block-diagonal q (only head h(d) column nonzero) moving.  All 9 tiles
    land in ONE 2KB PSUM bank.
  - a_all = exp(s_all - 3) in ONE activation per batch (global shift
    cancels in softmax).  Padded slots get a host-built poison K column
    with poison_d = -240*q_d/||q_head(d)||^2 so s_pad = -30 -> a ~ 0.
  - denom: one matmul ones^T @ a_all -> [1, ntb*8], then one DVE
    tensor_reduce over the tile axis -> dall[b*8:(b+1)*8].
  - merged[e, h] accumulated c-major (4 sequential accumulation groups in
    one PSUM bank): mc[:, c, :] += V_tile_chunk^T @ a_tile over 9 tiles.
  - head-diagonal extract via 8 small ACT/DVE copies into persistent
    mt_sb[p, c, b].
Tail (once per core): normalize by 1/denom (DVE), project through W_o^T
with float32r matmuls (1 cycle/row at N=512).
"""

import numpy as np

N_CORES = 8
BATCH = 256
NKEYS = 2048
EMB = 512
NH = 8
DH = 64
P = 128
KPAD = 1152          # padded packed-key count, 9 tiles of 128 (max count 1095)
NT = KPAD // P       # 9 key tiles per batch
NCH = EMB // P       # 4 contraction chunks over embedding dims
B_LOC = BATCH // N_CORES  # 32
NB9 = 18             # per-core batch positions 0..17 have 9 key tiles, rest 8
QSCALE = 1.0 / 8.0   # 1/sqrt(dh)
EXP_SHIFT = -3.0     # a = exp(s - 3); cancels in softmax, keeps a in bf16 range
POISON_S = -30.0     # target score for padded key slots
KVW = EMB + EMB      # 1024 bytes per (partition, tile): 512 K + 512 V
SEG = EMB            # one stream segment: 512 bytes per partition
# per-core batch tile counts and segment offsets into the packed kv stream:
# batch b contributes ntb K-segments then ntb V-segments, back to back.
_NTBS = [NT if b < NB9 else NT - 1 for b in range(B_LOC)]
_SOFF = [0]
for _n in _NTBS:
    _SOFF.append(_SOFF[-1] + 2 * _n)
NSEG = _SOFF[-1]     # 548 segments = 274 tiles of K + V


def build_nc(nb=B_LOC):
    """Build + compile the Bass program for one core processing `nb` batches."""
    import concourse.bass as bass
    import concourse.tile as tile
    from concourse import bacc, mybir

    f32 = mybir.dt.float32
    bf16 = mybir.dt.bfloat16
    f8e3 = mybir.dt.float8e3

    nc = bacc.Bacc(
        "TRN2",
        target_bir_lowering=False,
        debug=False,
        enable_asserts=True,
        num_devices=N_CORES,
    )
    kv = nc.dram_tensor("kv", [P, NSEG, SEG], f8e3, kind="ExternalInput").ap()
    qblk = nc.dram_tensor("qblk", [P, nb, NCH, NH], bf16, kind="ExternalInput").ap()
    wot = nc.dram_tensor("wot", [EMB, EMB], f32, kind="ExternalInput").ap()
    onesd = nc.dram_tensor("ones", [P, 1], bf16, kind="ExternalInput").ap()
    out = nc.dram_tensor("out", [nb, EMB], f32, kind="ExternalOutput").ap()

    with tile.TileContext(nc) as tc:
        _emit(tc, out, kv, qblk, wot, onesd, nb)
    nc.compile()
    return nc


def _emit(tc, out, kv, qblk, wot, onesd, nb):
    from contextlib import ExitStack

    from concourse import mybir

    f32 = mybir.dt.float32
    f32r = mybir.dt.float32r
    bf16 = mybir.dt.bfloat16
    nc = tc.nc
    f8e3 = mybir.dt.float8e3

    with ExitStack() as ctx:
        kvpool = ctx.enter_context(tc.tile_pool(name="kvpool", bufs=6))
        apool = ctx.enter_context(tc.tile_pool(name="apool", bufs=3))
        sp = ctx.enter_context(tc.tile_pool(name="sp", bufs=4))
        singles = ctx.enter_context(tc.tile_pool(name="singles", bufs=1))
        psum_s = ctx.enter_context(tc.tile_pool(name="psum_s", bufs=1, space="PSUM"))
        psum_mc = ctx.enter_context(tc.tile_pool(name="psum_mc", bufs=1, space="PSUM"))
        psum_d = ctx.enter_context(tc.tile_pool(name="psum_d", bufs=1, space="PSUM"))
        psum_o = ctx.enter_context(tc.tile_pool(name="psum_o", bufs=1, space="PSUM"))

        # ---- setup DMAs.  qblk + kv[b=0] gate the first matmul: qblk (small)
        # leads the sync ring, kv[0] the scalar ring.  ones (needed after the
        # first exp) goes via gpsimd SWDGE; wot rides the scalar ring after
        # the first few kv singles (needed only for the mid-loop f32r cast).
        qblk_sb = singles.tile([P, nb, NCH, NH], bf16)
        nc.gpsimd.dma_start(qblk_sb[:], qblk)
        ones_sb = singles.tile([P, 1], bf16)
        nc.gpsimd.dma_start(ones_sb[:], onesd)
        ebias_sb = singles.tile([P, 1], f32)
        nc.gpsimd.memset(ebias_sb[:], EXP_SHIFT)
        ones1_sb = singles.tile([1, P], f32)
        nc.gpsimd.memset(ones1_sb[:], 1.0)
        wot_sb = singles.tile([P, NCH, EMB], f32)
        # f32r-rounded copy of wot for the cheap (1 cycle/row) tail matmuls;
        # produced chunkwise on DVE mid-loop, off the critical path.
        wot_r = singles.tile([P, NCH, EMB], f32r)
        mt_n = singles.tile([P, NCH, nb], f32r)
        # unnormalized merged^T, built per batch by the extract copies:
        # mt_sb[p, c, b] = sum_k a_b[k, h] V_b[k, c*128+p], h = 2c + p//64
        mt_sb = singles.tile([P, NCH, nb], f32)
        # per-batch reciprocal denominators on one row: rall_sb[0, b*8 + h]
        rall_sb = singles.tile([1, nb * NH], f32)

        # kv DMA schedule over the packed segment stream: everything on the
        # sync (SP HWDGE) ring, whose queue carries nothing but DMAs.  The
        # ACT ring is NOT used for kv: dma_starts there would sit in the ACT
        # FIFO behind EXPs that transitively wait on those very DMAs.
        # Sizes escalate - batch 0 split K/V, singles, pairs, then quads -
        # so the pipeline ramps fast but steady state has few, fat DMAs.
        groups = [(0, 1), (1, 2), (2, 3), (3, 4), (4, 6), (6, 8)]
        while groups[-1][1] < nb:
            groups.append((groups[-1][1], min(groups[-1][1] + 3, nb)))
        wot_dma_emitted = False
        kvsb = [None] * nb  # per-batch (tile, base_segment) views
        for gi, (lo, hi) in enumerate(groups):
            nsg = _SOFF[hi] - _SOFF[lo]
            gtile = kvpool.tile(
                [P, nsg, SEG], f8e3, name=f"kvg{gi}", tag=f"kvg{hi - lo}",
                bufs=(4 if hi - lo == 1 else 2 if hi - lo == 2 else 3),
            )
            if gi == 0:
                # split batch 0 into K then V so scores start after the K half
                half = _NTBS[0]
                nc.sync.dma_start(gtile[:, 0:half], kv[:, 0:half])
                nc.sync.dma_start(gtile[:, half:nsg], kv[:, half:nsg])
            else:
                nc.sync.dma_start(
                    gtile[:], kv[:, _SOFF[lo] : _SOFF[hi]]
                )
            for b in range(lo, hi):
                kvsb[b] = (gtile, _SOFF[b] - _SOFF[lo])
            if not wot_dma_emitted and gi >= 3:
                nc.gpsimd.dma_start(
                    wot_sb[:], wot.rearrange("(c p) e -> p c e", p=P)
                )
                wot_dma_emitted = True

        def _half_tail(h):
            """Normalize + project batches [16h, 16h+16): half 0 is emitted
            mid-loop (hidden under the kv stream), half 1 at the end."""
            HB = nb // 2
            b0 = h * HB
            scaleB_ps = psum_d.tile([P, HB * NH], f32, name=f"scB{h}", tag="d")
            nc.tensor.matmul(
                scaleB_ps[:],
                ones1_sb[:],
                rall_sb[0:1, b0 * NH : (b0 + HB) * NH],
                start=True,
                stop=True,
            )
            scaleB_v = scaleB_ps.rearrange(
                "p (b c two) -> p c two b", b=HB, c=NCH, two=2
            )
            for hp in range(2):
                nc.vector.tensor_mul(
                    mt_n[hp * DH : (hp + 1) * DH, :, b0 : b0 + HB],
                    mt_sb[hp * DH : (hp + 1) * DH, :, b0 : b0 + HB],
                    scaleB_v[hp * DH : (hp + 1) * DH, :, hp, :],
                )
            out_ps = psum_o.tile([HB, EMB], f32, name=f"ops{h}", tag="ops", bufs=1)
            for c in range(NCH):
                nc.tensor.matmul(
                    out_ps[:],
                    mt_n[:, c, b0 : b0 + HB],
                    wot_r[:, c, :],
                    start=(c == 0),
                    stop=(c == NCH - 1),
                )
            out_sb = sp.tile([HB, EMB], f32, name=f"osb{h}", tag=f"osb{h}")
            nc.vector.tensor_copy(out_sb[:], out_ps[:])
            nc.sync.dma_start(out[b0 : b0 + HB], out_sb[:])

        for b in range(nb):
            # batches at positions >= NB9 have <= 1024 packed keys: skip the
            # all-padding 9th key tile entirely (less HBM traffic + PE work).
            ntb = _NTBS[b]
            gtile, base = kvsb[b]
            kvt = gtile[:, base : base + 2 * ntb]
            # wot -> f32r rounding, one chunk per iteration once wot has
            # certainly landed; DVE is otherwise idle here.
            if 16 <= b < 16 + NCH:
                cw = b - 16
                nc.vector.tensor_copy(wot_r[:, cw, :], wot_sb[:, cw, :])
            if b == 24:
                _half_tail(0)

            # ---- scores, split into two halves so the PE never waits on
            # the ACT exp: H0 = tiles 0..4, H1 = tiles 5..ntb-1.  Each half
            # is one single-buffered PSUM bank + one activation.
            JH = 5
            s_h0 = psum_s.tile([P, JH, NH], f32, name="s_h0", tag="s0")
            s_h1 = psum_s.tile([P, NT - JH, NH], f32, name="s_h1", tag="s1")
            a_h0 = apool.tile([P, JH, NH], bf16, name="a_h0", tag="a0", bufs=2)
            a_h1 = apool.tile([P, NT - JH, NH], bf16, name="a_h1", tag="a1", bufs=2)
            halves = [(0, JH, s_h0, a_h0), (JH, ntb, s_h1, a_h1)]
            for j0, j1, s_ps, a_sb in halves:
                for j in range(j0, j1):
                    for c in range(NCH):
                        nc.tensor.matmul(
                            s_ps[:, j - j0, :],
                            kvt[:, j, c * P : (c + 1) * P],
                            qblk_sb[:, b, c, :],
                            start=(c == 0),
                            stop=(c == NCH - 1),
                        )
                nc.scalar.activation(
                    a_sb[:, 0 : j1 - j0, :],
                    s_ps[:, 0 : j1 - j0, :],
                    mybir.ActivationFunctionType.Exp,
                    bias=ebias_sb[:],
                )

            def a_of(j):
                return a_h0[:, j, :] if j < JH else a_h1[:, j - JH, :]

            # ---- merged, per-tile with one accumulator bank per chunk
            mc = [
                psum_mc.tile([P, NH], f32, name=f"mc{c}", tag=f"mc{c}")
                for c in range(NCH)
            ]
            for j in range(ntb):
                for c in range(NCH):
                    nc.tensor.matmul(
                        mc[c][:],
                        kvt[:, ntb + j, c * P : (c + 1) * P],
                        a_of(j),
                        start=(j == 0),
                        stop=(j == ntb - 1),
                    )
            # ---- denominators: one matmul per half + one DVE reduce
            dsum = psum_d.tile([1, NT * NH], f32, tag="d")
            nc.tensor.matmul(
                dsum[0:1, 0 : JH * NH], ones_sb[:], a_h0[:], start=True, stop=True
            )
            nc.tensor.matmul(
                dsum[0:1, JH * NH : ntb * NH],
                ones_sb[:],
                a_h1[:, 0 : ntb - JH, :],
                start=True,
                stop=True,
            )
            dtmp = sp.tile([1, NH], f32, tag="dtmp")
            nc.vector.tensor_reduce(
                dtmp[:],
                dsum.rearrange("p (t h) -> p h t", t=NT, h=NH)[:, :, 0:ntb],
                axis=mybir.AxisListType.X,
                op=mybir.AluOpType.add,
            )
            nc.vector.reciprocal(rall_sb[0:1, b * NH : (b + 1) * NH], dtmp[:])
            # ---- extract the head-diagonal into SBUF: mt_sb[hp*64+p', c, b]
            # = mc[c][hp*64+p', 2c+hp]; split copies across ACT and DVE.
            for c in range(NCH):
                for hp in range(2):
                    h = 2 * c + hp
                    src = mc[c][hp * DH : (hp + 1) * DH, h : h + 1]
                    dst = mt_sb[hp * DH : (hp + 1) * DH, c, b : b + 1]
                    if h % 2 == 0:
                        nc.scalar.copy(dst, src)
                    else:
                        nc.vector.tensor_copy(dst, src)

        _half_tail(1)


def _prep_all(query, key, value, W_o, mask):
    """Host-side pack: gather unmasked K/V rows, pad, quantize, per-core shards.

    Returns (shards, perm): batches are permuted so every core's positions
    0..NB9-1 hold the batches with >1024 unmasked keys (9 key tiles); the rest
    need only 8. perm[i] = original batch index at packed position i.
    """
    import ml_dtypes

    bf16 = ml_dtypes.bfloat16
    f8e3 = ml_dtypes.float8_e3m4
    q_all = np.ascontiguousarray(query[:, 0, :], dtype=np.float32)  # [B, E]
    keep_all = ~mask[:, 0, :]  # True = attended
    counts_all = keep_all.sum(1).astype(np.int64)
    assert counts_all.max() <= KPAD, f"KPAD too small: {counts_all.max()}"

    heavy = np.flatnonzero(counts_all > (NT - 1) * P)
    light = np.flatnonzero(counts_all <= (NT - 1) * P)
    assert len(heavy) <= N_CORES * NB9, f"too many 9-tile batches: {len(heavy)}"
    pool9 = list(heavy)
    pool8 = list(light)
    per_core = [[] for _ in range(N_CORES)]
    for c in range(N_CORES):
        for _ in range(NB9):
            per_core[c].append(pool9.pop(0) if pool9 else pool8.pop(0))
    for c in range(N_CORES):
        for _ in range(B_LOC - NB9):
            per_core[c].append(pool8.pop(0))
    assert not pool9 and not pool8
    perm = np.array([b for core in per_core for b in core], dtype=np.int64)

    q = q_all[perm]
    keep = keep_all[perm]
    counts = counts_all[perm]

    gidx = np.empty((BATCH, KPAD), dtype=np.int64)
    for i in range(BATCH):
        idx = np.flatnonzero(keep[i])
        n = len(idx)
        gidx[i, :n] = idx
        gidx[i, n:] = idx[0] if n else 0
    rows = (gidx + perm[:, None] * NKEYS).reshape(-1)

    # K: gather packed rows, poison the padded slots, fp8 e3m4.
    Kp = key.reshape(-1, EMB)[rows].reshape(BATCH, KPAD, EMB)
    qh2 = (q.reshape(BATCH, NH, DH) ** 2).sum(-1)  # [B, H] per-head |q|^2
    poison = (POISON_S / QSCALE) * q / np.repeat(qh2, DH, axis=1)  # [B, E]
    # e3m4 tops out at 15.5: clip so the fp8 cast can't overflow to inf.
    # The clipped dot is still <= -28 for every (batch, head) -> exp ~ 0.
    np.clip(poison, -14.0, 14.0, out=poison)
    padpos = np.arange(KPAD)[None, :] >= counts[:, None]
    bi, ji = np.nonzero(padpos)
    Kp[bi, ji] = poison[bi]
    # tile-major, c-major within tile: kpt[p, b, t, c*128+j2] =
    # Kp[b, t*128+j2, c*128+p] so any leading-tile slice is one contiguous
    # per-partition DMA together with the V half.
    kpt = (
        Kp.astype(f8e3)
        .reshape(BATCH, NT, P, NCH, P)
        .transpose(4, 0, 1, 3, 2)
        .reshape(P, BATCH, NT, EMB)
    )

    # V: gather packed rows (pad rows harmless: their weight is exp(-33)),
    # fp8 e3m4 (max 15.5 >> |V|max ~5.5), vp[p, b, t, e] = Vp[b, t*128+p, e].
    Vp = value.reshape(-1, EMB)[rows].reshape(BATCH, KPAD, EMB)
    vp = Vp.astype(f8e3).reshape(BATCH, NT, P, EMB).transpose(2, 0, 1, 3)

    # packed per-core stream: for each local batch, ntb K segments then ntb
    # V segments, concatenated with no inter-batch holes -> every DMA group
    # is one contiguous per-partition run.
    kv_streams = []
    for core in range(N_CORES):
        segs = []
        for i, ntb in enumerate(_NTBS):
            gb = core * B_LOC + i
            segs.append(kpt[:, gb, 0:ntb])
            segs.append(vp[:, gb, 0:ntb])
        kv_streams.append(np.ascontiguousarray(np.concatenate(segs, axis=1)))

    # qblk [P, B, NCH, NH]: block-diagonal scaled q. Column h = 2c + (p>=64)
    # holds q[c*128+p]/8; all other columns zero.
    qs = (q * np.float32(QSCALE)).reshape(BATCH, NCH, 2, DH)
    qblk = np.zeros((BATCH, NCH, P, NH), np.float32)
    for c in range(NCH):
        for hp in range(2):
            qblk[:, c, hp * DH : (hp + 1) * DH, 2 * c + hp] = qs[:, c, hp, :]
    qblk = np.ascontiguousarray(qblk.transpose(2, 0, 1, 3).astype(bf16))

    wot = np.ascontiguousarray(W_o.T).astype(np.float32)
    ones = np.ones((P, 1), dtype=bf16)

    shards = []
    for c in range(N_CORES):
        lo, hi = c * B_LOC, (c + 1) * B_LOC
        shards.append(
            {
                "kv": kv_streams[c],
                "qblk": np.ascontiguousarray(qblk[:, lo:hi]),
                "wot": wot,
                "ones": ones,
            }
        )
    return shards, perm


_NC_CACHE = {}


def _get_nc():
    if "nc" not in _NC_CACHE:
        _NC_CACHE["nc"] = build_nc()
    return _NC_CACHE["nc"]


def kernel(query, key, value, W_o, mask):
    from concourse import bass_utils

    query = np.asarray(query, dtype=np.float32)
    key = np.asarray(key, dtype=np.float32)
    value = np.asarray(value, dtype=np.float32)
    W_o = np.asarray(W_o, dtype=np.float32)
    mask = np.asarray(mask)

    nc = _get_nc()
    in_maps, perm = _prep_all(query, key, value, W_o, mask)
    res = bass_utils.run_bass_kernel_spmd(
        nc, in_maps, core_ids=list(range(N_CORES)), trace=False
    )
    out_perm = np.concatenate([res.results[c]["out"] for c in range(N_CORES)], axis=0)
    out = np.empty_like(out_perm)
    out[perm] = out_perm
    return out.reshape(BATCH, 1, EMB).astype(np.float32, copy=False)


if __name__ == "__main__":
    # smoke: build the program only
    nc = build_nc()
    print("built + compiled OK; instructions:", len(list(nc.all_instructions())))


# revision 23
# speedup vs baseline: 1.0714x; 1.0714x over previous
"""Trainium2 Bass kernel: masked multi-head decode attention + output projection.

Problem (hardcoded): query [256,1,512] f32, key/value [256,2048,512] f32,
W_o [512,512] f32, mask [256,1,2048] bool (True = excluded).
out = Linear(W_o) o MHA(query, key, value, mask), 8 heads, dh=64.

Strategy: data-parallel over batch on 8 NeuronCores (32 batches/core).
~50% of keys are masked out and contribute nothing, so the host packs only
the unmasked K/V rows per batch (padded to KPAD=1152 = 9 tiles of 128) and
stores BOTH K and V in fp8 e3m4 — ~7x less HBM traffic than the dense f32
layout. K and V for one batch travel in a single combined DMA.

Per batch on-core:
  - kv[p, t, 0:512]  = K tile t, c-major chunks: kv[p,t,c*128+j] = K[key
    t*128+j, dim c*128+p]; kv[p, t, 512:1024] = V row t*128+p.
  - scores: s_all[k, j*8+h] accumulated over 4 chunk matmuls per key tile
    with the 128-col K chunk stationary (fp8 FWL load) and the
    block-diagonal q (only head h(d) column nonzero) moving.  All 9 tiles
    land in ONE 2KB PSUM bank.
  - a_all = exp(s_all - 3) in ONE activation per batch (global shift
    cancels in softmax).  Padded slots get a host-built poison K column
    with poison_d = -240*q_d/||q_head(d)||^2 so s_pad = -30 -> a ~ 0.
  - denom: one matmul ones^T @ a_all -> [1, ntb*8], then one DVE
    tensor_reduce over the tile axis -> dall[b*8:(b+1)*8].
  - merged[e, h] accumulated c-major (4 sequential accumulation groups in
    one PSUM bank): mc[:, c, :] += V_tile_chunk^T @ a_tile over 9 tiles.
  - head-diagonal extract via 8 small ACT/DVE copies into persistent
    mt_sb[p, c, b].
Tail (once per core): normalize by 1/denom (DVE), project through W_o^T
with float32r matmuls (1 cycle/row at N=512).
"""

import numpy as np

N_CORES = 8
BATCH = 256
NKEYS = 2048
EMB = 512
NH = 8
DH = 64
P = 128
KPAD = 1152          # padded packed-key count, 9 tiles of 128 (max count 1095)
NT = KPAD // P       # 9 key tiles per batch
NCH = EMB // P       # 4 contraction chunks over embedding dims
B_LOC = BATCH // N_CORES  # 32
NB9 = 18             # per-core batch positions 0..17 have 9 key tiles, rest 8
QSCALE = 1.0 / 8.0   # 1/sqrt(dh)
EXP_SHIFT = -3.0     # a = exp(s - 3); cancels in softmax, keeps a in bf16 range
POISON_S = -30.0     # target score for padded key slots
KVW = EMB + EMB      # 1024 bytes per (partition, tile): 512 K + 512 V
SEG = EMB            # one stream segment: 512 bytes per partition
# per-core batch tile counts and segment offsets into the packed kv stream:
# batch b contributes ntb K-segments then ntb V-segments, back to back.
_NTBS = [NT if b < NB9 else NT - 1 for b in range(B_LOC)]
_SOFF = [0]
for _n in _NTBS:
    _SOFF.append(_SOFF[-1] + 2 * _n)
NSEG = _SOFF[-1]     # 548 segments = 274 tiles of K + V


def build_nc(nb=B_LOC):
    """Build + compile the Bass program for one core processing `nb` batches."""
    import concourse.bass as bass
    import concourse.tile as tile
    from concourse import bacc, mybir

    f32 = mybir.dt.float32
    bf16 = mybir.dt.bfloat16
    f8e3 = mybir.dt.float8e3

    nc = bacc.Bacc(
        "TRN2",
        target_bir_lowering=False,
        debug=False,
        enable_asserts=True,
        num_devices=N_CORES,
    )
    kv = nc.dram_tensor("kv", [P, NSEG, SEG], f8e3, kind="ExternalInput").ap()
    qblk = nc.dram_tensor("qblk", [P, nb, NCH, NH], bf16, kind="ExternalInput").ap()
    wot = nc.dram_tensor("wot", [EMB, EMB], f32, kind="ExternalInput").ap()
    onesd = nc.dram_tensor("ones", [P, 1], bf16, kind="ExternalInput").ap()
    out = nc.dram_tensor("out", [nb, EMB], f32, kind="ExternalOutput").ap()

    with tile.TileContext(nc) as tc:
        _emit(tc, out, kv, qblk, wot, onesd, nb)
    nc.compile()
    return nc


def _emit(tc, out, kv, qblk, wot, onesd, nb):
    from contextlib import ExitStack

    from concourse import mybir

    f32 = mybir.dt.float32
    f32r = mybir.dt.float32r
    bf16 = mybir.dt.bfloat16
    nc = tc.nc
    f8e3 = mybir.dt.float8e3

    with ExitStack() as ctx:
        kvpool = ctx.enter_context(tc.tile_pool(name="kvpool", bufs=6))
        apool = ctx.enter_context(tc.tile_pool(name="apool", bufs=3))
        sp = ctx.enter_context(tc.tile_pool(name="sp", bufs=4))
        singles = ctx.enter_context(tc.tile_pool(name="singles", bufs=1))
        psum_s = ctx.enter_context(tc.tile_pool(name="psum_s", bufs=1, space="PSUM"))
        psum_mc = ctx.enter_context(tc.tile_pool(name="psum_mc", bufs=1, space="PSUM"))
        psum_d = ctx.enter_context(tc.tile_pool(name="psum_d", bufs=1, space="PSUM"))
        psum_o = ctx.enter_context(tc.tile_pool(name="psum_o", bufs=1, space="PSUM"))

        # ---- setup DMAs.  qblk + kv[b=0] gate the first matmul: qblk (small)
        # leads the sync ring, kv[0] the scalar ring.  ones (needed after the
        # first exp) goes via gpsimd SWDGE; wot rides the scalar ring after
        # the first few kv singles (needed only for the mid-loop f32r cast).
        qblk_sb = singles.tile([P, nb, NCH, NH], bf16)
        nc.gpsimd.dma_start(qblk_sb[:], qblk)
        ones_sb = singles.tile([P, 1], bf16)
        nc.gpsimd.dma_start(ones_sb[:], onesd)
        ebias_sb = singles.tile([P, 1], f32)
        nc.gpsimd.memset(ebias_sb[:], EXP_SHIFT)
        ones1_sb = singles.tile([1, P], f32)
        nc.gpsimd.memset(ones1_sb[:], 1.0)
        wot_sb = singles.tile([P, NCH, EMB], f32)
        # f32r-rounded copy of wot for the cheap (1 cycle/row) tail matmuls;
        # produced chunkwise on DVE mid-loop, off the critical path.
        wot_r = singles.tile([P, NCH, EMB], f32r)
        mt_n = singles.tile([P, NCH, nb], f32r)
        # unnormalized merged^T, built per batch by the extract copies:
        # mt_sb[p, c, b] = sum_k a_b[k, h] V_b[k, c*128+p], h = 2c + p//64
        mt_sb = singles.tile([P, NCH, nb], f32)
        # per-batch reciprocal denominators on one row: rall_sb[0, b*8 + h]
        rall_sb = singles.tile([1, nb * NH], f32)

        # kv DMA schedule over the packed segment stream: everything on the
        # sync (SP HWDGE) ring, whose queue carries nothing but DMAs.  The
        # ACT ring is NOT used for kv: dma_starts there would sit in the ACT
        # FIFO behind EXPs that transitively wait on those very DMAs.
        # Sizes escalate - batch 0 split K/V, singles, pairs, then quads -
        # so the pipeline ramps fast but steady state has few, fat DMAs.
        groups = [(0, 1), (1, 2), (2, 3), (3, 4), (4, 6), (6, 8)]
        while groups[-1][1] < nb:
            groups.append((groups[-1][1], min(groups[-1][1] + 3, nb)))
        wot_dma_emitted = False
        kvsb = [None] * nb  # per-batch (tile, base_segment) views
        for gi, (lo, hi) in enumerate(groups):
            nsg = _SOFF[hi] - _SOFF[lo]
            gtile = kvpool.tile(
                [P, nsg, SEG], f8e3, name=f"kvg{gi}", tag=f"kvg{hi - lo}",
                bufs=(4 if hi - lo == 1 else 2 if hi - lo == 2 else 3),
            )
            if gi == 0:
                # split batch 0 into K then V so scores start after the K half
                half = _NTBS[0]
                nc.sync.dma_start(gtile[:, 0:half], kv[:, 0:half])
                nc.sync.dma_start(gtile[:, half:nsg], kv[:, half:nsg])
            else:
                nc.sync.dma_start(
                    gtile[:], kv[:, _SOFF[lo] : _SOFF[hi]]
                )
            for b in range(lo, hi):
                kvsb[b] = (gtile, _SOFF[b] - _SOFF[lo])
            if not wot_dma_emitted and gi >= 3:
                nc.gpsimd.dma_start(
                    wot_sb[:], wot.rearrange("(c p) e -> p c e", p=P)
                )
                wot_dma_emitted = True

        def _half_tail(h):
            """Normalize + project batches [16h, 16h+16): half 0 is emitted
            mid-loop (hidden under the kv stream), half 1 at the end."""
            HB = nb // 2
            b0 = h * HB
            scaleB_ps = psum_d.tile([P, HB * NH], f32, name=f"scB{h}", tag="d")
            nc.tensor.matmul(
                scaleB_ps[:],
                ones1_sb[:],
                rall_sb[0:1, b0 * NH : (b0 + HB) * NH],
                start=True,
                stop=True,
            )
            scaleB_v = scaleB_ps.rearrange(
                "p (b c two) -> p c two b", b=HB, c=NCH, two=2
            )
            for hp in range(2):
                nc.vector.tensor_mul(
                    mt_n[hp * DH : (hp + 1) * DH, :, b0 : b0 + HB],
                    mt_sb[hp * DH : (hp + 1) * DH, :, b0 : b0 + HB],
                    scaleB_v[hp * DH : (hp + 1) * DH, :, hp, :],
                )
            out_ps = psum_o.tile([HB, EMB], f32, name=f"ops{h}", tag="ops", bufs=1)
            for c in range(NCH):
                nc.tensor.matmul(
                    out_ps[:],
                    mt_n[:, c, b0 : b0 + HB],
                    wot_r[:, c, :],
                    start=(c == 0),
                    stop=(c == NCH - 1),
                )
            out_sb = sp.tile([HB, EMB], f32, name=f"osb{h}", tag=f"osb{h}")
            nc.vector.tensor_copy(out_sb[:], out_ps[:])
            nc.sync.dma_start(out[b0 : b0 + HB], out_sb[:])

        for b in range(nb):
            # batches at positions >= NB9 have <= 1024 packed keys: skip the
            # all-padding 9th key tile entirely (less HBM traffic + PE work).
            ntb = _NTBS[b]
            gtile, base = kvsb[b]
            kvt = gtile[:, base : base + 2 * ntb]
            # wot -> f32r rounding, one chunk per iteration once wot has
            # certainly landed; DVE is otherwise idle here.
            if 16 <= b < 16 + NCH:
                cw = b - 16
                nc.vector.tensor_copy(wot_r[:, cw, :], wot_sb[:, cw, :])
            if b == 24:
                _half_tail(0)

            # ---- scores, split into two halves so the PE never waits on
            # the ACT exp: H0 = tiles 0..4, H1 = tiles 5..ntb-1.  Each half
            # is one single-buffered PSUM bank + one activation.
            JH = 5
            s_h0 = psum_s.tile([P, JH, NH], f32, name="s_h0", tag="s0")
            s_h1 = psum_s.tile([P, NT - JH, NH], f32, name="s_h1", tag="s1")
            a_h0 = apool.tile([P, JH, NH], bf16, name="a_h0", tag="a0", bufs=2)
            a_h1 = apool.tile([P, NT - JH, NH], bf16, name="a_h1", tag="a1", bufs=2)
            halves = [(0, JH, s_h0, a_h0), (JH, ntb, s_h1, a_h1)]
            for j0, j1, s_ps, a_sb in halves:
                for j in range(j0, j1):
                    for c in range(NCH):
                        nc.tensor.matmul(
                            s_ps[:, j - j0, :],
                            kvt[:, j, c * P : (c + 1) * P],
                            qblk_sb[:, b, c, :],
                            start=(c == 0),
                            stop=(c == NCH - 1),
                        )
                nc.scalar.activation(
                    a_sb[:, 0 : j1 - j0, :],
                    s_ps[:, 0 : j1 - j0, :],
                    mybir.ActivationFunctionType.Exp,
                    bias=ebias_sb[:],
                )

            def a_of(j):
                return a_h0[:, j, :] if j < JH else a_h1[:, j - JH, :]

            # ---- merged, c-major: 4 sequential accumulation groups in ONE
            # bank (back-to-back 27ns LDW/MM cadence); the a operand switches
            # from a_h0 to a_h1 mid-group, so early merged MMs only wait on
            # the first half-exp.
            mcb = psum_mc.tile([P, NCH, NH], f32, tag="mc", bufs=2)
            for c in range(NCH):
                for j in range(ntb):
                    nc.tensor.matmul(
                        mcb[:, c, :],
                        kvt[:, ntb + j, c * P : (c + 1) * P],
                        a_of(j),
                        start=(j == 0),
                        stop=(j == ntb - 1),
                    )
            # ---- denominators: one matmul per half + one DVE reduce
            dsum = psum_d.tile([1, NT * NH], f32, tag="d")
            nc.tensor.matmul(
                dsum[0:1, 0 : JH * NH], ones_sb[:], a_h0[:], start=True, stop=True
            )
            nc.tensor.matmul(
                dsum[0:1, JH * NH : ntb * NH],
                ones_sb[:],
                a_h1[:, 0 : ntb - JH, :],
                start=True,
                stop=True,
            )
            dtmp = sp.tile([1, NH], f32, tag="dtmp")
            nc.vector.tensor_reduce(
                dtmp[:],
                dsum.rearrange("p (t h) -> p h t", t=NT, h=NH)[:, :, 0:ntb],
                axis=mybir.AxisListType.X,
                op=mybir.AluOpType.add,
            )
            nc.vector.reciprocal(rall_sb[0:1, b * NH : (b + 1) * NH], dtmp[:])
            # ---- extract the head-diagonal into SBUF: mt_sb[hp*64+p', c, b]
            # = mc[c][hp*64+p', 2c+hp]; split copies across ACT and DVE.
            for c in range(NCH):
                for hp in range(2):
                    h = 2 * c + hp
                    src = mcb[hp * DH : (hp + 1) * DH, c, h : h + 1]
                    dst = mt_sb[hp * DH : (hp + 1) * DH, c, b : b + 1]
                    if h % 2 == 0:
                        nc.scalar.copy(dst, src)
                    else:
                        nc.vector.tensor_copy(dst, src)

        _half_tail(1)


def _prep_all(query, key, value, W_o, mask):
    """Host-side pack: gather unmasked K/V rows, pad, quantize, per-core shards.

    Returns (shards, perm): batches are permuted so every core's positions
    0..NB9-1 hold the batches with >1024 unmasked keys (9 key tiles); the rest
    need only 8. perm[i] = original batch index at packed position i.
    """
    import ml_dtypes

    bf16 = ml_dtypes.bfloat16
    f8e3 = ml_dtypes.float8_e3m4
    q_all = np.ascontiguousarray(query[:, 0, :], dtype=np.float32)  # [B, E]
    keep_all = ~mask[:, 0, :]  # True = attended
    counts_all = keep_all.sum(1).astype(np.int64)
    assert counts_all.max() <= KPAD, f"KPAD too small: {counts_all.max()}"

    heavy = np.flatnonzero(counts_all > (NT - 1) * P)
    light = np.flatnonzero(counts_all <= (NT - 1) * P)
    assert len(heavy) <= N_CORES * NB9, f"too many 9-tile batches: {len(heavy)}"
    pool9 = list(heavy)
    pool8 = list(light)
    per_core = [[] for _ in range(N_CORES)]
    for c in range(N_CORES):
        for _ in range(NB9):
            per_core[c].append(pool9.pop(0) if pool9 else pool8.pop(0))
    for c in range(N_CORES):
        for _ in range(B_LOC - NB9):
            per_core[c].append(pool8.pop(0))
    assert not pool9 and not pool8
    perm = np.array([b for core in per_core for b in core], dtype=np.int64)

    q = q_all[perm]
    keep = keep_all[perm]
    counts = counts_all[perm]

    gidx = np.empty((BATCH, KPAD), dtype=np.int64)
    for i in range(BATCH):
        idx = np.flatnonzero(keep[i])
        n = len(idx)
        gidx[i, :n] = idx
        gidx[i, n:] = idx[0] if n else 0
    rows = (gidx + perm[:, None] * NKEYS).reshape(-1)

    # K: gather packed rows, poison the padded slots, fp8 e3m4.
    Kp = key.reshape(-1, EMB)[rows].reshape(BATCH, KPAD, EMB)
    qh2 = (q.reshape(BATCH, NH, DH) ** 2).sum(-1)  # [B, H] per-head |q|^2
    poison = (POISON_S / QSCALE) * q / np.repeat(qh2, DH, axis=1)  # [B, E]
    # e3m4 tops out at 15.5: clip so the fp8 cast can't overflow to inf.
    # The clipped dot is still <= -28 for every (batch, head) -> exp ~ 0.
    np.clip(poison, -14.0, 14.0, out=poison)
    padpos = np.arange(KPAD)[None, :] >= counts[:, None]
    bi, ji = np.nonzero(padpos)
    Kp[bi, ji] = poison[bi]
    # tile-major, c-major within tile: kpt[p, b, t, c*128+j2] =
    # Kp[b, t*128+j2, c*128+p] so any leading-tile slice is one contiguous
    # per-partition DMA together with the V half.
    kpt = (
        Kp.astype(f8e3)
        .reshape(BATCH, NT, P, NCH, P)
        .transpose(4, 0, 1, 3, 2)
        .reshape(P, BATCH, NT, EMB)
    )

    # V: gather packed rows (pad rows harmless: their weight is exp(-33)),
    # fp8 e3m4 (max 15.5 >> |V|max ~5.5), vp[p, b, t, e] = Vp[b, t*128+p, e].
    Vp = value.reshape(-1, EMB)[rows].reshape(BATCH, KPAD, EMB)
    vp = Vp.astype(f8e3).reshape(BATCH, NT, P, EMB).transpose(2, 0, 1, 3)

    # packed per-core stream: for each local batch, ntb K segments then ntb
    # V segments, concatenated with no inter-batch holes -> every DMA group
    # is one contiguous per-partition run.
    kv_streams = []
    for core in range(N_CORES):
        segs = []
        for i, ntb in enumerate(_NTBS):
            gb = core * B_LOC + i
            segs.append(kpt[:, gb, 0:ntb])
            segs.append(vp[:, gb, 0:ntb])
        kv_streams.append(np.ascontiguousarray(np.concatenate(segs, axis=1)))

    # qblk [P, B, NCH, NH]: block-diagonal scaled q. Column h = 2c + (p>=64)
    # holds q[c*128+p]/8; all other columns zero.
    qs = (q * np.float32(QSCALE)).reshape(BATCH, NCH, 2, DH)
    qblk = np.zeros((BATCH, NCH, P, NH), np.float32)
    for c in range(NCH):
        for hp in range(2):
            qblk[:, c, hp * DH : (hp + 1) * DH, 2 * c + hp] = qs[:, c, hp, :]
    qblk = np.ascontiguousarray(qblk.transpose(2, 0, 1, 3).astype(bf16))

    wot = np.ascontiguousarray(W_o.T).astype(np.float32)
    ones = np.ones((P, 1), dtype=bf16)

    shards = []
    for c in range(N_CORES):
        lo, hi = c * B_LOC, (c + 1) * B_LOC
        shards.append(
            {
                "kv": kv_streams[c],
                "qblk": np.ascontiguousarray(qblk[:, lo:hi]),
                "wot": wot,
                "ones": ones,
            }
        )
    return shards, perm


_NC_CACHE = {}


def _get_nc():
    if "nc" not in _NC_CACHE:
        _NC_CACHE["nc"] = build_nc()
    return _NC_CACHE["nc"]


def kernel(query, key, value, W_o, mask):
    from concourse import bass_utils

    query = np.asarray(query, dtype=np.float32)
    key = np.asarray(key, dtype=np.float32)
    value = np.asarray(value, dtype=np.float32)
    W_o = np.asarray(W_o, dtype=np.float32)
    mask = np.asarray(mask)

    nc = _get_nc()
    in_maps, perm = _prep_all(query, key, value, W_o, mask)
    res = bass_utils.run_bass_kernel_spmd(
        nc, in_maps, core_ids=list(range(N_CORES)), trace=False
    )
    out_perm = np.concatenate([res.results[c]["out"] for c in range(N_CORES)], axis=0)
    out = np.empty_like(out_perm)
    out[perm] = out_perm
    return out.reshape(BATCH, 1, EMB).astype(np.float32, copy=False)


if __name__ == "__main__":
    # smoke: build the program only
    nc = build_nc()
    print("built + compiled OK; instructions:", len(list(nc.all_instructions())))


# revision 24
# speedup vs baseline: 1.0917x; 1.0190x over previous
"""Trainium2 Bass kernel: masked multi-head decode attention + output projection.

Problem (hardcoded): query [256,1,512] f32, key/value [256,2048,512] f32,
W_o [512,512] f32, mask [256,1,2048] bool (True = excluded).
out = Linear(W_o) o MHA(query, key, value, mask), 8 heads, dh=64.

Strategy: data-parallel over batch on 8 NeuronCores (32 batches/core).
~50% of keys are masked out and contribute nothing, so the host packs only
the unmasked K/V rows per batch (padded to KPAD=1152 = 9 tiles of 128) and
stores BOTH K and V in fp8 e3m4 — ~7x less HBM traffic than the dense f32
layout. K and V for one batch travel in a single combined DMA.

Per batch on-core:
  - kv[p, t, 0:512]  = K tile t, c-major chunks: kv[p,t,c*128+j] = K[key
    t*128+j, dim c*128+p]; kv[p, t, 512:1024] = V row t*128+p.
  - scores: s_all[k, j*8+h] accumulated over 4 chunk matmuls per key tile
    with the 128-col K chunk stationary (fp8 FWL load) and the
    block-diagonal q (only head h(d) column nonzero) moving.  All 9 tiles
    land in ONE 2KB PSUM bank.
  - a_all = exp(s_all - 3) in ONE activation per batch (global shift
    cancels in softmax).  Padded slots get a host-built poison K column
    with poison_d = -240*q_d/||q_head(d)||^2 so s_pad = -30 -> a ~ 0.
  - denom: one matmul ones^T @ a_all -> [1, ntb*8], then one DVE
    tensor_reduce over the tile axis -> dall[b*8:(b+1)*8].
  - merged[e, h] accumulated c-major (4 sequential accumulation groups in
    one PSUM bank): mc[:, c, :] += V_tile_chunk^T @ a_tile over 9 tiles.
  - head-diagonal extract via 8 small ACT/DVE copies into persistent
    mt_sb[p, c, b].
Tail (once per core): normalize by 1/denom (DVE), project through W_o^T
with float32r matmuls (1 cycle/row at N=512).
"""

import numpy as np

N_CORES = 8
BATCH = 256
NKEYS = 2048
EMB = 512
NH = 8
DH = 64
P = 128
KPAD = 1152          # padded packed-key count, 9 tiles of 128 (max count 1095)
NT = KPAD // P       # 9 key tiles per batch
NCH = EMB // P       # 4 contraction chunks over embedding dims
B_LOC = BATCH // N_CORES  # 32
NB9 = 18             # per-core batch positions 0..17 have 9 key tiles, rest 8
QSCALE = 1.0 / 8.0   # 1/sqrt(dh)
EXP_SHIFT = -3.0     # a = exp(s - 3); cancels in softmax, keeps a in bf16 range
POISON_S = -30.0     # target score for padded key slots
KVW = EMB + EMB      # 1024 bytes per (partition, tile): 512 K + 512 V
SEG = EMB            # one stream segment: 512 bytes per partition
# per-core batch tile counts and segment offsets into the packed kv stream:
# batch b contributes ntb K-segments then ntb V-segments, back to back.
_NTBS = [NT if b < NB9 else NT - 1 for b in range(B_LOC)]
_SOFF = [0]
for _n in _NTBS:
    _SOFF.append(_SOFF[-1] + 2 * _n)
NSEG = _SOFF[-1]     # 548 segments = 274 tiles of K + V


def build_nc(nb=B_LOC):
    """Build + compile the Bass program for one core processing `nb` batches."""
    import concourse.bass as bass
    import concourse.tile as tile
    from concourse import bacc, mybir

    f32 = mybir.dt.float32
    bf16 = mybir.dt.bfloat16
    f8e3 = mybir.dt.float8e3

    nc = bacc.Bacc(
        "TRN2",
        target_bir_lowering=False,
        debug=False,
        enable_asserts=True,
        num_devices=N_CORES,
    )
    kv = nc.dram_tensor("kv", [P, NSEG, SEG], f8e3, kind="ExternalInput").ap()
    qblk = nc.dram_tensor("qblk", [P, nb, NCH, NH], bf16, kind="ExternalInput").ap()
    wot = nc.dram_tensor("wot", [EMB, EMB], f32, kind="ExternalInput").ap()
    onesd = nc.dram_tensor("ones", [P, 1], bf16, kind="ExternalInput").ap()
    out = nc.dram_tensor("out", [nb, EMB], f32, kind="ExternalOutput").ap()

    with tile.TileContext(nc) as tc:
        _emit(tc, out, kv, qblk, wot, onesd, nb)
    nc.compile()
    return nc


def _emit(tc, out, kv, qblk, wot, onesd, nb):
    from contextlib import ExitStack

    from concourse import mybir

    f32 = mybir.dt.float32
    f32r = mybir.dt.float32r
    bf16 = mybir.dt.bfloat16
    nc = tc.nc
    f8e3 = mybir.dt.float8e3

    with ExitStack() as ctx:
        kvpool = ctx.enter_context(tc.tile_pool(name="kvpool", bufs=6))
        apool = ctx.enter_context(tc.tile_pool(name="apool", bufs=3))
        sp = ctx.enter_context(tc.tile_pool(name="sp", bufs=4))
        singles = ctx.enter_context(tc.tile_pool(name="singles", bufs=1))
        psum_s = ctx.enter_context(tc.tile_pool(name="psum_s", bufs=1, space="PSUM"))
        psum_mc = ctx.enter_context(tc.tile_pool(name="psum_mc", bufs=1, space="PSUM"))
        psum_d = ctx.enter_context(tc.tile_pool(name="psum_d", bufs=1, space="PSUM"))
        psum_o = ctx.enter_context(tc.tile_pool(name="psum_o", bufs=1, space="PSUM"))

        # ---- setup DMAs.  qblk + kv[b=0] gate the first matmul: qblk (small)
        # leads the sync ring, kv[0] the scalar ring.  ones (needed after the
        # first exp) goes via gpsimd SWDGE; wot rides the scalar ring after
        # the first few kv singles (needed only for the mid-loop f32r cast).
        qblk_sb = singles.tile([P, nb, NCH, NH], bf16)
        nc.gpsimd.dma_start(qblk_sb[:], qblk)
        ones_sb = singles.tile([P, 1], bf16)
        nc.gpsimd.dma_start(ones_sb[:], onesd)
        ebias_sb = singles.tile([P, 1], f32)
        nc.gpsimd.memset(ebias_sb[:], EXP_SHIFT)
        ones1_sb = singles.tile([1, P], f32)
        nc.gpsimd.memset(ones1_sb[:], 1.0)
        wot_sb = singles.tile([P, NCH, EMB], f32)
        # f32r-rounded copy of wot for the cheap (1 cycle/row) tail matmuls;
        # produced chunkwise on DVE mid-loop, off the critical path.
        wot_r = singles.tile([P, NCH, EMB], f32r)
        mt_n = singles.tile([P, NCH, nb], f32r)
        # unnormalized merged^T, built per batch by the extract copies:
        # mt_sb[p, c, b] = sum_k a_b[k, h] V_b[k, c*128+p], h = 2c + p//64
        mt_sb = singles.tile([P, NCH, nb], f32)
        # per-batch reciprocal denominators on one row: rall_sb[0, b*8 + h]
        rall_sb = singles.tile([1, nb * NH], f32)

        # kv DMA schedule over the packed segment stream: everything on the
        # sync (SP HWDGE) ring, whose queue carries nothing but DMAs.  The
        # ACT ring is NOT used for kv: dma_starts there would sit in the ACT
        # FIFO behind EXPs that transitively wait on those very DMAs.
        # Sizes escalate - batch 0 split K/V, singles, pairs, then quads -
        # so the pipeline ramps fast but steady state has few, fat DMAs.
        groups = [(0, 1), (1, 2), (2, 3), (3, 4), (4, 6), (6, 8)]
        while groups[-1][1] < nb:
            groups.append((groups[-1][1], min(groups[-1][1] + 3, nb)))
        wot_dma_emitted = False
        kvsb = [None] * nb  # per-batch (tile, base_segment) views
        for gi, (lo, hi) in enumerate(groups):
            nsg = _SOFF[hi] - _SOFF[lo]
            gtile = kvpool.tile(
                [P, nsg, SEG], f8e3, name=f"kvg{gi}", tag=f"kvg{hi - lo}",
                bufs=(4 if hi - lo == 1 else 2 if hi - lo == 2 else 3),
            )
            if gi == 0:
                # split batch 0 into K then V so scores start after the K half
                half = _NTBS[0]
                nc.sync.dma_start(gtile[:, 0:half], kv[:, 0:half])
                nc.sync.dma_start(gtile[:, half:nsg], kv[:, half:nsg])
            else:
                nc.sync.dma_start(
                    gtile[:], kv[:, _SOFF[lo] : _SOFF[hi]]
                )
            for b in range(lo, hi):
                kvsb[b] = (gtile, _SOFF[b] - _SOFF[lo])
            if not wot_dma_emitted and gi >= 3:
                nc.gpsimd.dma_start(
                    wot_sb[:], wot.rearrange("(c p) e -> p c e", p=P)
                )
                wot_dma_emitted = True

        def _half_tail(h):
            """Normalize + project batches [16h, 16h+16): half 0 is emitted
            mid-loop (hidden under the kv stream), half 1 at the end."""
            HB = nb // 2
            b0 = h * HB
            scaleB_ps = psum_d.tile([P, HB * NH], f32, name=f"scB{h}", tag="d")
            nc.tensor.matmul(
                scaleB_ps[:],
                ones1_sb[:],
                rall_sb[0:1, b0 * NH : (b0 + HB) * NH],
                start=True,
                stop=True,
            )
            scaleB_v = scaleB_ps.rearrange(
                "p (b c two) -> p c two b", b=HB, c=NCH, two=2
            )
            for hp in range(2):
                nc.vector.tensor_mul(
                    mt_n[hp * DH : (hp + 1) * DH, :, b0 : b0 + HB],
                    mt_sb[hp * DH : (hp + 1) * DH, :, b0 : b0 + HB],
                    scaleB_v[hp * DH : (hp + 1) * DH, :, hp, :],
                )
            out_ps = psum_o.tile([HB, EMB], f32, name=f"ops{h}", tag="ops", bufs=1)
            for c in range(NCH):
                nc.tensor.matmul(
                    out_ps[:],
                    mt_n[:, c, b0 : b0 + HB],
                    wot_r[:, c, :],
                    start=(c == 0),
                    stop=(c == NCH - 1),
                )
            out_sb = sp.tile([HB, EMB], f32, name=f"osb{h}", tag=f"osb{h}")
            nc.vector.tensor_copy(out_sb[:], out_ps[:])
            nc.sync.dma_start(out[b0 : b0 + HB], out_sb[:])

        for b in range(nb):
            # batches at positions >= NB9 have <= 1024 packed keys: skip the
            # all-padding 9th key tile entirely (less HBM traffic + PE work).
            ntb = _NTBS[b]
            gtile, base = kvsb[b]
            kvt = gtile[:, base : base + 2 * ntb]
            # wot -> f32r rounding, one chunk per iteration once wot has
            # certainly landed; DVE is otherwise idle here.
            if 16 <= b < 16 + NCH:
                cw = b - 16
                nc.vector.tensor_copy(wot_r[:, cw, :], wot_sb[:, cw, :])
            if b == 24:
                _half_tail(0)

            # ---- scores for all tiles into one PSUM bank: s_all[k, j, h]
            s_all = psum_s.tile([P, NT, NH], f32, tag="s0", bufs=2)
            for j in range(ntb):
                for c in range(NCH):
                    nc.tensor.matmul(
                        s_all[:, j, :],
                        kvt[:, j, c * P : (c + 1) * P],
                        qblk_sb[:, b, c, :],
                        start=(c == 0),
                        stop=(c == NCH - 1),
                    )
            # ---- one exp per batch (ACT), bias = -3 per partition
            a_all = apool.tile([P, NT, NH], bf16, tag="a", bufs=3)
            nc.scalar.activation(
                a_all[:, 0:ntb, :],
                s_all[:, 0:ntb, :],
                mybir.ActivationFunctionType.Exp,
                bias=ebias_sb[:],
            )
            # ---- merged, c-major: 4 sequential accumulation groups, 1 bank
            mcb = psum_mc.tile([P, NCH, NH], f32, tag="mc", bufs=2)
            for c in range(NCH):
                for j in range(ntb):
                    nc.tensor.matmul(
                        mcb[:, c, :],
                        kvt[:, ntb + j, c * P : (c + 1) * P],
                        a_all[:, j, :],
                        start=(j == 0),
                        stop=(j == ntb - 1),
                    )
            # ---- denominators: one matmul + one DVE reduce over the tile axis
            dsum = psum_d.tile([1, NT * NH], f32, tag="d")
            nc.tensor.matmul(
                dsum[0:1, 0 : ntb * NH],
                ones_sb[:],
                a_all[:, 0:ntb, :],
                start=True,
                stop=True,
            )
            dtmp = sp.tile([1, NH], f32, tag="dtmp")
            nc.vector.tensor_reduce(
                dtmp[:],
                dsum.rearrange("p (t h) -> p h t", t=NT, h=NH)[:, :, 0:ntb],
                axis=mybir.AxisListType.X,
                op=mybir.AluOpType.add,
            )
            nc.vector.reciprocal(rall_sb[0:1, b * NH : (b + 1) * NH], dtmp[:])
            # ---- extract the head-diagonal into SBUF: mt_sb[hp*64+p', c, b]
            # = mc[c][hp*64+p', 2c+hp]; split copies across ACT and DVE.
            for c in range(NCH):
                for hp in range(2):
                    h = 2 * c + hp
                    src = mcb[hp * DH : (hp + 1) * DH, c, h : h + 1]
                    dst = mt_sb[hp * DH : (hp + 1) * DH, c, b : b + 1]
                    if h % 2 == 0:
                        nc.scalar.copy(dst, src)
                    else:
                        nc.vector.tensor_copy(dst, src)

        _half_tail(1)


def _prep_all(query, key, value, W_o, mask):
    """Host-side pack: gather unmasked K/V rows, pad, quantize, per-core shards.

    Returns (shards, perm): batches are permuted so every core's positions
    0..NB9-1 hold the batches with >1024 unmasked keys (9 key tiles); the rest
    need only 8. perm[i] = original batch index at packed position i.
    """
    import ml_dtypes

    bf16 = ml_dtypes.bfloat16
    f8e3 = ml_dtypes.float8_e3m4
    q_all = np.ascontiguousarray(query[:, 0, :], dtype=np.float32)  # [B, E]
    keep_all = ~mask[:, 0, :]  # True = attended
    counts_all = keep_all.sum(1).astype(np.int64)
    assert counts_all.max() <= KPAD, f"KPAD too small: {counts_all.max()}"

    heavy = np.flatnonzero(counts_all > (NT - 1) * P)
    light = np.flatnonzero(counts_all <= (NT - 1) * P)
    assert len(heavy) <= N_CORES * NB9, f"too many 9-tile batches: {len(heavy)}"
    pool9 = list(heavy)
    pool8 = list(light)
    per_core = [[] for _ in range(N_CORES)]
    for c in range(N_CORES):
        for _ in range(NB9):
            per_core[c].append(pool9.pop(0) if pool9 else pool8.pop(0))
    for c in range(N_CORES):
        for _ in range(B_LOC - NB9):
            per_core[c].append(pool8.pop(0))
    assert not pool9 and not pool8
    perm = np.array([b for core in per_core for b in core], dtype=np.int64)

    q = q_all[perm]
    keep = keep_all[perm]
    counts = counts_all[perm]

    gidx = np.empty((BATCH, KPAD), dtype=np.int64)
    for i in range(BATCH):
        idx = np.flatnonzero(keep[i])
        n = len(idx)
        gidx[i, :n] = idx
        gidx[i, n:] = idx[0] if n else 0
    rows = (gidx + perm[:, None] * NKEYS).reshape(-1)

    # K: gather packed rows, poison the padded slots, fp8 e3m4.
    Kp = key.reshape(-1, EMB)[rows].reshape(BATCH, KPAD, EMB)
    qh2 = (q.reshape(BATCH, NH, DH) ** 2).sum(-1)  # [B, H] per-head |q|^2
    poison = (POISON_S / QSCALE) * q / np.repeat(qh2, DH, axis=1)  # [B, E]
    # e3m4 tops out at 15.5: clip so the fp8 cast can't overflow to inf.
    # The clipped dot is still <= -28 for every (batch, head) -> exp ~ 0.
    np.clip(poison, -14.0, 14.0, out=poison)
    padpos = np.arange(KPAD)[None, :] >= counts[:, None]
    bi, ji = np.nonzero(padpos)
    Kp[bi, ji] = poison[bi]
    # tile-major, c-major within tile: kpt[p, b, t, c*128+j2] =
    # Kp[b, t*128+j2, c*128+p] so any leading-tile slice is one contiguous
    # per-partition DMA together with the V half.
    kpt = (
        Kp.astype(f8e3)
        .reshape(BATCH, NT, P, NCH, P)
        .transpose(4, 0, 1, 3, 2)
        .reshape(P, BATCH, NT, EMB)
    )

    # V: gather packed rows (pad rows harmless: their weight is exp(-33)),
    # fp8 e3m4 (max 15.5 >> |V|max ~5.5), vp[p, b, t, e] = Vp[b, t*128+p, e].
    Vp = value.reshape(-1, EMB)[rows].reshape(BATCH, KPAD, EMB)
    vp = Vp.astype(f8e3).reshape(BATCH, NT, P, EMB).transpose(2, 0, 1, 3)

    # packed per-core stream: for each local batch, ntb K segments then ntb
    # V segments, concatenated with no inter-batch holes -> every DMA group
    # is one contiguous per-partition run.
    kv_streams = []
    for core in range(N_CORES):
        segs = []
        for i, ntb in enumerate(_NTBS):
            gb = core * B_LOC + i
            segs.append(kpt[:, gb, 0:ntb])
            segs.append(vp[:, gb, 0:ntb])
        kv_streams.append(np.ascontiguousarray(np.concatenate(segs, axis=1)))

    # qblk [P, B, NCH, NH]: block-diagonal scaled q. Column h = 2c + (p>=64)
    # holds q[c*128+p]/8; all other columns zero.
    qs = (q * np.float32(QSCALE)).reshape(BATCH, NCH, 2, DH)
    qblk = np.zeros((BATCH, NCH, P, NH), np.float32)
    for c in range(NCH):
        for hp in range(2):
            qblk[:, c, hp * DH : (hp + 1) * DH, 2 * c + hp] = qs[:, c, hp, :]
    qblk = np.ascontiguousarray(qblk.transpose(2, 0, 1, 3).astype(bf16))

    wot = np.ascontiguousarray(W_o.T).astype(np.float32)
    ones = np.ones((P, 1), dtype=bf16)

    shards = []
    for c in range(N_CORES):
        lo, hi = c * B_LOC, (c + 1) * B_LOC
        shards.append(
            {
                "kv": kv_streams[c],
                "qblk": np.ascontiguousarray(qblk[:, lo:hi]),
                "wot": wot,
                "ones": ones,
            }
        )
    return shards, perm


_NC_CACHE = {}


def _get_nc():
    if "nc" not in _NC_CACHE:
        _NC_CACHE["nc"] = build_nc()
    return _NC_CACHE["nc"]


def kernel(query, key, value, W_o, mask):
    from concourse import bass_utils

    query = np.asarray(query, dtype=np.float32)
    key = np.asarray(key, dtype=np.float32)
    value = np.asarray(value, dtype=np.float32)
    W_o = np.asarray(W_o, dtype=np.float32)
    mask = np.asarray(mask)

    nc = _get_nc()
    in_maps, perm = _prep_all(query, key, value, W_o, mask)
    res = bass_utils.run_bass_kernel_spmd(
        nc, in_maps, core_ids=list(range(N_CORES)), trace=False
    )
    out_perm = np.concatenate([res.results[c]["out"] for c in range(N_CORES)], axis=0)
    out = np.empty_like(out_perm)
    out[perm] = out_perm
    return out.reshape(BATCH, 1, EMB).astype(np.float32, copy=False)


if __name__ == "__main__":
    # smoke: build the program only
    nc = build_nc()
    print("built + compiled OK; instructions:", len(list(nc.all_instructions())))
